# revision 16
# baseline (speedup 1.0000x reference)
"""Segment-mean (word-pooling) kernel for Trainium2, 8 NeuronCores.

Problem: hidden_states [16, 4096, 768] f32, word_ids [16, 4096] i32
(non-decreasing per row, -1 = special token). Output [16, 2048, 768] f32:
mean of each word's subword embeddings; words with no tokens -> 0.

Strategy: pure data parallelism, 2 samples per core. Per sample, the
segment-mean is computed as a banded one-hot matmul on the PE:
  out[w, h] = sum_s onehot[s, w] * (1/count[w]) * x[s, h]
Tokens are processed in 32 k-tiles of 128; since word ids are
non-decreasing, each k-tile only touches a <=128-wide band of words, so
each k-tile contributes 1-2 matmuls into 128-word output windows
accumulated in PSUM. The one-hot (scaled by per-token reciprocal counts,
computed on host) is built on the vector engine with a single fused
is_equal*mult tensor_scalar op per k-tile against an iota ramp.

The SPMD program is identical on all 8 cores; the (k-tile, window)
pair structure is the union over samples, so per-core data that doesn't
touch a scheduled pair just contributes a zero one-hot block.
"""

import numpy as np

B, S, H = 16, 4096, 768
NUM_WORDS = S // 2  # 2048
N_CORES = 8
SPC = B // N_CORES  # samples per core = 2
P = 128
KT = S // P  # 32 k-tiles per sample
NW = NUM_WORDS // P  # 16 output windows per sample
NSPLITS = ((0, 512), (512, 768))  # matmul free-dim splits of H


def _plan(word_ids: np.ndarray):
    """Per-slot union plan. For each slot (0/1) and k-tile t: the window
    span [minwin, maxwin] over that slot's 8 samples; per window j the
    sorted member k-tiles. Returns (spans, members) per slot."""
    word_ids = np.minimum(word_ids, NUM_WORDS - 1)
    plans = []
    for slot in range(SPC):
        wid = word_ids[slot::SPC]  # the 8 samples this slot sees
        minwin = np.full(KT, NW, np.int64)
        maxwin = np.full(KT, -1, np.int64)
        for b in range(wid.shape[0]):
            row = wid[b]
            for t in range(KT):
                w = row[t * P : (t + 1) * P]
                w = w[w >= 0]
                if w.size:
                    minwin[t] = min(minwin[t], w.min() // P)
                    maxwin[t] = max(maxwin[t], w.max() // P)
        members = {j: [] for j in range(NW)}
        spans = []
        for t in range(KT):
            if maxwin[t] < 0:  # no valid token anywhere (can't happen)
                spans.append((0, 0))
                continue
            spans.append((int(minwin[t]), int(maxwin[t])))
            for j in range(int(minwin[t]), int(maxwin[t]) + 1):
                members[j].append(t)
        plans.append((spans, members))
    return plans


def _liveness(plans, in_group):
    """Max number of simultaneously-live x DMA groups / onehot tiles over
    the per-window emission order, across slots. A pool needs at least
    this many bufs or slot reuse can deadlock the DMA ring."""
    max_live_g, max_live_oh = 0, 0
    for spans, members in plans:
        first_g, last_g, first_oh, last_oh = {}, {}, {}, {}
        for j in range(NW):
            for t in members[j]:
                g = t // in_group
                first_g.setdefault(g, j)
                last_g[g] = j
                first_oh.setdefault(t, j)
                last_oh[t] = j
        for j in range(NW):
            live_g = sum(1 for g in first_g if first_g[g] <= j <= last_g[g])
            live_oh = sum(1 for t in first_oh if first_oh[t] <= j <= last_oh[t])
            max_live_g = max(max_live_g, live_g)
            max_live_oh = max(max_live_oh, live_oh)
    return max_live_g, max_live_oh


def _recip_counts(word_ids: np.ndarray) -> np.ndarray:
    """Per-token 1/count(word) as f32; 0 for special (-1) tokens."""
    r = np.zeros((B, S), np.float32)
    for b in range(B):
        wid = word_ids[b]
        valid = wid >= 0
        counts = np.bincount(wid[valid], minlength=NUM_WORDS)
        r[b, valid] = (1.0 / counts[wid[valid]]).astype(np.float32)
    return r


def _build(
    plans,
    reps=1,
    dyn_reps=1,
    do_mm=True,
    do_out=True,
    do_in=True,
    do_oh=True,
    do_wid=True,
    x_bufs=8,
    oh_bufs=8,
    ev_bufs=4,
    ps_bufs=3,
    in_group=4,
    out_group=2,
    in_alt=False,
    out_engine="scalar",
    in_dtype="f16",
    out_dtype="f16",
    n_splits=None,
    ev_engine="scalar",
    in_layout="rowmajor",
    staggered=False,
):
    """Build + compile the SPMD Bass program. reps>1 unrolls the whole
    body; dyn_reps>1 wraps it in a hardware For loop — both only used
    for amortized wall-clock timing. do_* flags ablate kernel stages
    for benchmarking (outputs are wrong when any is False)."""
    from contextlib import nullcontext
    import concourse.bacc as bacc
    import concourse.tile as tile
    from concourse import mybir

    nc = bacc.Bacc(
        "TRN2",
        target_bir_lowering=False,
        debug=False,
        enable_asserts=False,
        num_devices=N_CORES,
    )
    f32 = mybir.dt.float32
    fin = mybir.dt.float16 if in_dtype == "f16" else f32
    fout = mybir.dt.float16 if out_dtype == "f16" else f32
    if in_layout == "pmajor":
        x = nc.dram_tensor(
            "x", [SPC * KT // in_group, P, in_group * H], fin, kind="ExternalInput"
        ).ap()
    else:
        x = nc.dram_tensor("x", [SPC * S, H], fin, kind="ExternalInput").ap()
    widf = nc.dram_tensor("widf", [SPC, P, KT], f32, kind="ExternalInput").ap()
    rcp = nc.dram_tensor("rcp", [SPC, P, KT], f32, kind="ExternalInput").ap()
    y = nc.dram_tensor("y", [SPC * NUM_WORDS, H], fout, kind="ExternalOutput").ap()

    IOTA_W = NUM_WORDS + 2 * P  # ramp long enough for any window pair
    max_span = max(
        (jhi - jlo + 1) for spans, _ in plans for (jlo, jhi) in spans
    )

    # Size pools from plan liveness; degenerate plans (heavily overlapping
    # window k-ranges) fall back to a bounded reload mode, otherwise pool
    # slot reuse can deadlock the DMA ring.
    in_b = 2 if in_dtype == "f16" else 4
    live_g, live_oh = _liveness(plans, in_group)
    need_x, need_oh = live_g + 3, live_oh + 3
    x_bytes = need_x * in_group * H * in_b
    oh_bytes = need_oh * max_span * P * in_b
    safe = x_bytes + oh_bytes > 150 * 1024
    if not safe:
        x_bufs = max(x_bufs, need_x)
        oh_bufs = max(oh_bufs, need_oh)

    with tile.TileContext(nc) as tc:
        with (
            tc.tile_pool(name="const", bufs=1) as const_pool,
            tc.tile_pool(name="xin", bufs=x_bufs) as x_pool,
            tc.tile_pool(name="oh", bufs=oh_bufs) as oh_pool,
            tc.tile_pool(name="ev", bufs=ev_bufs) as ev_pool,
            tc.tile_pool(name="psum", bufs=ps_bufs, space="PSUM") as psum_pool,
        ):
            iota_i = const_pool.tile([P, IOTA_W], mybir.dt.int32)
            nc.gpsimd.iota(iota_i[:], pattern=[[1, IOTA_W]], base=0, channel_multiplier=0)
            iota_f = const_pool.tile([P, IOTA_W], f32)
            nc.vector.tensor_copy(out=iota_f[:], in_=iota_i[:])

            IG, OG = in_group, out_group
            out_eng = nc.sync if out_engine == "sync" else nc.scalar
            ev_eng = nc.vector if ev_engine == "vector" else nc.scalar
            splits = NSPLITS if n_splits is None else n_splits

            def emit(rep):
                for slot in range(SPC):
                    spans, members = plans[slot]
                    if do_wid:
                        wid_t = const_pool.tile(
                            [P, KT], f32, name=f"wid_{rep}_{slot}", tag=f"wid{slot}"
                        )
                        nc.scalar.dma_start(out=wid_t[:], in_=widf[slot, :, :])
                        rcp_t = const_pool.tile(
                            [P, KT], f32, name=f"rcp_{rep}_{slot}", tag=f"rcp{slot}"
                        )
                        nc.scalar.dma_start(out=rcp_t[:], in_=rcp[slot, :, :])

                    xg_tiles = {}
                    oh_tiles = {}

                    def get_x(t):
                        g, a = divmod(t, IG)
                        if g not in xg_tiles:
                            xt = x_pool.tile(
                                [P, IG, H], fin, name=f"xt_{rep}_{slot}_{g}", tag="xt"
                            )
                            if do_in:
                                if in_layout == "pmajor":
                                    src = x[slot * (KT // IG) + g, :, :].rearrange(
                                        "p (a h) -> p a h", a=IG
                                    )
                                else:
                                    r0 = slot * S + g * IG * P
                                    src = x[r0 : r0 + IG * P, :].rearrange(
                                        "(a p) h -> p a h", p=P
                                    )
                                eng = (
                                    nc.scalar
                                    if (in_alt and g % 2 == 1)
                                    else nc.sync
                                )
                                eng.dma_start(out=xt[:], in_=src)
                            xg_tiles[g] = xt
                        return xg_tiles[g][:, t % IG, :]

                    def get_oh(t):
                        if t not in oh_tiles:
                            jlo, jhi = spans[t]
                            wspan = (jhi - jlo + 1) * P
                            oh = oh_pool.tile(
                                [P, max_span * P],
                                fin,
                                name=f"oh_{rep}_{slot}_{t}",
                                tag="oh",
                            )
                            nc.vector.tensor_scalar(
                                out=oh[:, :wspan],
                                in0=iota_f[:, jlo * P : jlo * P + wspan],
                                scalar1=wid_t[:, t : t + 1],
                                scalar2=rcp_t[:, t : t + 1],
                                op0=mybir.AluOpType.is_equal,
                                op1=mybir.AluOpType.mult,
                            )
                            oh_tiles[t] = oh
                        return oh_tiles[t]

                    og_tile = [None]

                    for j in range(NW):
                        if (do_mm or do_out) and j % OG == 0:
                            og_tile[0] = ev_pool.tile(
                                [P, OG, H], fout, name=f"out_{rep}_{slot}_{j}", tag="out"
                            )
                        out_sb = og_tile[0][:, j % OG, :] if (do_mm or do_out) else None
                        ks = members[j]
                        if not do_mm:
                            for t in ks:
                                if do_in:
                                    get_x(t)
                                if do_oh:
                                    get_oh(t)
                        if not ks:
                            if out_sb is not None:
                                nc.vector.memset(out_sb, 0.0)
                        elif not do_mm:
                            if do_out:
                                nc.gpsimd.memset(out_sb, 0.0)
                        else:
                            ps = psum_pool.tile(
                                [P, H], f32, name=f"ps_{rep}_{slot}_{j}", tag="ps"
                            )
                            for ki, t in enumerate(ks):
                                if safe:
                                    xs = x_pool.tile(
                                        [P, 1, H], fin,
                                        name=f"xs_{rep}_{slot}_{j}_{t}", tag="xt",
                                    )
                                    r0 = slot * S + t * P
                                    nc.sync.dma_start(
                                        out=xs[:], in_=x[r0 : r0 + P, :].rearrange(
                                            "(a p) h -> p a h", p=P
                                        )
                                    )
                                    xt = xs[:, 0, :]
                                    oh = oh_pool.tile(
                                        [P, P], fin,
                                        name=f"ohs_{rep}_{slot}_{j}_{t}", tag="oh",
                                    )
                                    nc.vector.tensor_scalar(
                                        out=oh[:, :],
                                        in0=iota_f[:, j * P : (j + 1) * P],
                                        scalar1=wid_t[:, t : t + 1],
                                        scalar2=rcp_t[:, t : t + 1],
                                        op0=mybir.AluOpType.is_equal,
                                        op1=mybir.AluOpType.mult,
                                    )
                                    off = 0
                                else:
                                    xt = get_x(t)
                                    oh = get_oh(t)
                                    off = (j - spans[t][0]) * P
                                for lo, hi in splits:
                                    nc.tensor.matmul(
                                        out=ps[:, lo:hi],
                                        lhsT=oh[:, off : off + P],
                                        rhs=xt[:, lo:hi],
                                        start=(ki == 0),
                                        stop=(ki == len(ks) - 1),
                                    )
                            if ev_engine == "vector":
                                nc.vector.tensor_copy(out=out_sb, in_=ps[:])
                            else:
                                nc.scalar.copy(out=out_sb, in_=ps[:])
                        if do_out and j % OG == OG - 1:
                            r0 = slot * NUM_WORDS + (j - OG + 1) * P
                            dst = y[r0 : r0 + OG * P, :].rearrange(
                                "(a p) h -> p a h", p=P
                            )
                            oe = (
                                (nc.scalar if (j // OG) % 2 == 0 else nc.sync)
                                if out_engine == "alt"
                                else out_eng
                            )
                            oe.dma_start(out=dst, in_=og_tile[0][:])

            loop_cm = (
                tc.For_i(0, dyn_reps, 1, staggered_reset=staggered)
                if dyn_reps > 1
                else nullcontext()
            )
            with loop_cm:
                for rep in range(reps):
                    emit(rep)

    nc.compile()
    return nc


def _prep_inputs(hidden_states, word_ids, in_dtype="f16", in_layout="rowmajor",
                 in_group=4):
    np_in = np.float16 if in_dtype == "f16" else np.float32
    hs = np.ascontiguousarray(np.asarray(hidden_states, dtype=np_in))
    wid = np.minimum(np.asarray(word_ids, dtype=np.int32), NUM_WORDS - 1)
    assert hs.shape == (B, S, H) and wid.shape == (B, S)
    r = _recip_counts(wid)
    # [B, S] -> [B, P, KT]: element (p, t) = token t*P + p
    widf = np.ascontiguousarray(
        wid.astype(np.float32).reshape(B, KT, P).transpose(0, 2, 1)
    )
    rt = np.ascontiguousarray(r.reshape(B, KT, P).transpose(0, 2, 1))
    in_maps = []
    for c in range(N_CORES):
        sl = slice(c * SPC, (c + 1) * SPC)
        if in_layout == "pmajor":
            IG = in_group
            xc = np.ascontiguousarray(
                hs[sl]
                .reshape(SPC, KT // IG, IG, P, H)
                .transpose(0, 1, 3, 2, 4)
                .reshape(SPC * KT // IG, P, IG * H)
            )
        else:
            xc = hs[sl].reshape(SPC * S, H)
        in_maps.append({"x": xc, "widf": widf[sl], "rcp": rt[sl]})
    return in_maps


def _plan2(word_ids: np.ndarray):
    """Per-slot union plan, pair-flattened. For each slot: the ordered pair
    list [(j, t), ...] (windows ascending, k-tiles ascending within a
    window) and members[j] = sorted k-tiles feeding window j. The pair list
    is the union over the 8 samples mapped to that slot, so one SPMD
    program fits all cores; blocks a core's sample doesn't touch produce
    all-zero one-hots and contribute nothing."""
    wid = np.minimum(np.asarray(word_ids, np.int32), NUM_WORDS - 1)
    plans = []
    for slot in range(SPC):
        w = wid[slot::SPC]
        members = {j: [] for j in range(NW)}
        for t in range(KT):
            seg = w[:, t * P : (t + 1) * P]
            v = seg[seg >= 0]
            jlo, jhi = int(v.min()) // P, int(v.max()) // P
            for j in range(jlo, jhi + 1):
                members[j].append(t)
        pairs = [(j, t) for j in range(NW) for t in members[j]]
        plans.append((pairs, members))
    return plans


def _x_liveness2(members, in_group):
    """Max simultaneously-live x groups over the window emission order."""
    first_g, last_g = {}, {}
    for j in range(NW):
        for t in members[j]:
            g = t // in_group
            first_g.setdefault(g, j)
            last_g[g] = j
    return max(
        sum(1 for g in first_g if first_g[g] <= j <= last_g[g]) for j in range(NW)
    )


def _prep_inputs2(hidden_states, word_ids, plans, pairs_max, in_group=8):
    """Per-core inputs for _build2. shifted[slot][p, i] = word_id of token
    (t_i*P + p) minus j_i*128 for pair i=(j_i,t_i): in [0,128) iff the
    token's word is in pair i's window (specials/-1 and out-of-window
    tokens fall outside automatically). rcpw[slot][p, j] = 1/count(word
    j*128+p), 0 for empty words. x is partition-major: [group, p,
    IG*H] so each partition's bytes per load are one contiguous run."""
    hs = np.ascontiguousarray(np.asarray(hidden_states, dtype=np.float16))
    wid = np.minimum(np.asarray(word_ids, dtype=np.int32), NUM_WORDS - 1)
    assert hs.shape == (B, S, H) and wid.shape == (B, S)
    IG = in_group
    in_maps = []
    for c in range(N_CORES):
        shifted = np.full((SPC, P, pairs_max), -2048, np.float16)
        rcpw = np.zeros((SPC, P, NW), np.float32)
        for slot in range(SPC):
            pairs, _ = plans[slot]
            w = wid[c * SPC + slot]
            for i, (j, t) in enumerate(pairs):
                shifted[slot, :, i] = (w[t * P : (t + 1) * P] - j * P).astype(
                    np.float16
                )
            valid = w >= 0
            counts = np.bincount(w[valid], minlength=NUM_WORDS)
            r = np.zeros(NUM_WORDS, np.float32)
            nz = counts > 0
            r[nz] = 1.0 / counts[nz]
            rcpw[slot] = r.reshape(NW, P).T
        xc = np.ascontiguousarray(
            hs[c * SPC : (c + 1) * SPC]
            .reshape(SPC, KT // IG, IG, P, H)
            .transpose(0, 1, 3, 2, 4)
            .reshape(SPC * (KT // IG), P, IG * H)
        )
        in_maps.append({"x": xc, "shifted": shifted, "rcpw": rcpw})
    return in_maps


def _build2(
    plans,
    reps=1,
    dyn_reps=1,
    do_mm=True,
    do_out=True,
    do_in=True,
    do_oh=True,
    x_bufs=6,
    oh_bufs=3,
    ev_bufs=4,
    ps_bufs=4,
    in_group=8,
    out_group=4,
    out_engine="scalar",
    ev_engine="both",
    oh_chunk=16,
    staggered=False,
):
    """v2: one-hot blocks for ALL pairs of a slot are built with a single
    tensor_tensor is_equal against a broadcast 0..127 ramp; 1/count scaling
    is applied per word row during PSUM evacuation (per-partition scale),
    so no per-k-tile vector ops remain."""
    from contextlib import nullcontext
    import concourse.bacc as bacc
    import concourse.tile as tile
    from concourse import mybir

    nc = bacc.Bacc(
        "TRN2",
        target_bir_lowering=False,
        debug=False,
        enable_asserts=False,
        num_devices=N_CORES,
    )
    f32 = mybir.dt.float32
    f16 = mybir.dt.float16
    i32 = mybir.dt.int32

    IG, OG = in_group, out_group
    pairs_max = max(len(pairs) for pairs, _ in plans)
    x = nc.dram_tensor(
        "x", [SPC * (KT // IG), P, IG * H], f16, kind="ExternalInput"
    ).ap()
    shifted = nc.dram_tensor(
        "shifted", [SPC, P, pairs_max], f16, kind="ExternalInput"
    ).ap()
    rcpw = nc.dram_tensor("rcpw", [SPC, P, NW], f32, kind="ExternalInput").ap()
    y = nc.dram_tensor(
        "y", [SPC * (NW // OG), P, OG * H], f16, kind="ExternalOutput"
    ).ap()
    live = max(_x_liveness2(members, IG) for _, members in plans)
    x_bufs = max(x_bufs, live + 2)
    assert x_bufs * IG * H * 2 * P <= 14 * 1024 * 1024, "x pool too big"

    with tile.TileContext(nc) as tc:
        with (
            tc.tile_pool(name="const", bufs=1) as const_pool,
            tc.tile_pool(name="sw", bufs=4) as sw_pool,
            tc.tile_pool(name="xin", bufs=x_bufs) as x_pool,
            tc.tile_pool(name="oh", bufs=oh_bufs) as oh_pool,
            tc.tile_pool(name="ev", bufs=ev_bufs) as ev_pool,
            tc.tile_pool(name="psum", bufs=ps_bufs, space="PSUM") as psum_pool,
        ):
            ramp_i = const_pool.tile([P, P], i32)
            nc.gpsimd.iota(ramp_i[:], pattern=[[1, P]], base=0, channel_multiplier=0)
            ramp_f = const_pool.tile([P, P], f16)
            nc.vector.tensor_copy(out=ramp_f[:], in_=ramp_i[:])

            def emit(rep):
                for slot in range(SPC):
                    pairs, members = plans[slot]
                    npair = len(pairs)
                    sh_t = sw_pool.tile(
                        [P, pairs_max], f16, name=f"sh_{rep}_{slot}", tag="sh"
                    )
                    rw_t = sw_pool.tile(
                        [P, NW], f32, name=f"rw_{rep}_{slot}", tag="rw"
                    )
                    if do_oh or do_mm:
                        nc.scalar.dma_start(out=sh_t[:], in_=shifted[slot, :, :])
                        nc.scalar.dma_start(out=rw_t[:], in_=rcpw[slot, :, :])

                    oh_all = oh_pool.tile(
                        [P, pairs_max * P], f16, name=f"oh_{rep}_{slot}", tag="oh"
                    )
                    if do_oh or do_mm:
                        for c0 in range(0, npair, oh_chunk):
                            c1 = min(c0 + oh_chunk, npair)
                            nchunk = c1 - c0
                            nc.vector.tensor_tensor(
                                out=oh_all[:, c0 * P : c1 * P].rearrange(
                                    "p (i w) -> p i w", w=P
                                ),
                                in0=sh_t[:, c0:c1].unsqueeze(2).broadcast_to(
                                    (P, nchunk, P)
                                ),
                                in1=ramp_f[:].unsqueeze(1).broadcast_to(
                                    (P, nchunk, P)
                                ),
                                op=mybir.AluOpType.is_equal,
                            )

                    xg_tiles = {}

                    def get_x(t):
                        g, a = divmod(t, IG)
                        if g not in xg_tiles:
                            xt = x_pool.tile(
                                [P, IG, H], f16, name=f"xt_{rep}_{slot}_{g}", tag="xt"
                            )
                            if do_in:
                                src = x[slot * (KT // IG) + g, :, :].rearrange(
                                    "p (a h) -> p a h", a=IG
                                )
                                nc.sync.dma_start(out=xt[:], in_=src)
                            xg_tiles[g] = xt
                        return xg_tiles[g][:, t % IG, :]

                    og_tile = [None]
                    pair_idx = 0
                    for j in range(NW):
                        if (do_mm or do_out) and j % OG == 0:
                            og_tile[0] = ev_pool.tile(
                                [P, OG, H], f16, name=f"out_{rep}_{slot}_{j}",
                                tag="out",
                            )
                        out_sb = (
                            og_tile[0][:, j % OG, :] if (do_mm or do_out) else None
                        )
                        ks = members[j]
                        if not do_mm:
                            pair_idx += len(ks)
                            if do_in:
                                for t in ks:
                                    get_x(t)
                            if do_out and out_sb is not None:
                                nc.gpsimd.memset(out_sb, 0.0)
                        elif not ks:
                            nc.vector.memset(out_sb, 0.0)
                        else:
                            ps = psum_pool.tile(
                                [P, H], f32, name=f"ps_{rep}_{slot}_{j}", tag="ps"
                            )
                            for ki, t in enumerate(ks):
                                xt = get_x(t)
                                lhsT = oh_all[:, pair_idx * P : (pair_idx + 1) * P]
                                pair_idx += 1
                                for lo, hi in NSPLITS:
                                    nc.tensor.matmul(
                                        out=ps[:, lo:hi],
                                        lhsT=lhsT,
                                        rhs=xt[:, lo:hi],
                                        start=(ki == 0),
                                        stop=(ki == len(ks) - 1),
                                    )
                            scale = rw_t[:, j : j + 1]
                            use_vec = ev_engine == "vector" or (
                                ev_engine == "both" and j % 2 == 0
                            )
                            if use_vec:
                                nc.vector.tensor_scalar(
                                    out=out_sb,
                                    in0=ps[:],
                                    scalar1=scale,
                                    scalar2=None,
                                    op0=mybir.AluOpType.mult,
                                )
                            else:
                                nc.scalar.activation(
                                    out_sb,
                                    ps[:],
                                    mybir.ActivationFunctionType.Copy,
                                    scale=scale,
                                )
                        if do_out and j % OG == OG - 1:
                            dst = y[slot * (NW // OG) + j // OG, :, :].rearrange(
                                "p (a h) -> p a h", a=OG
                            )
                            oe = (
                                (nc.scalar if (j // OG) % 2 == 0 else nc.sync)
                                if out_engine == "alt"
                                else (nc.sync if out_engine == "sync" else nc.scalar)
                            )
                            oe.dma_start(out=dst, in_=og_tile[0][:])

            loop_cm = (
                tc.For_i(0, dyn_reps, 1, staggered_reset=staggered)
                if dyn_reps > 1
                else nullcontext()
            )
            with loop_cm:
                for rep in range(reps):
                    emit(rep)

    nc.compile()
    return nc


KERNEL_IG = 8
KERNEL_OG = 4


def kernel(hidden_states, word_ids):
    import concourse.bass_utils as bass_utils

    wid = np.asarray(word_ids, dtype=np.int32)
    plans = _plan2(wid)
    nc = _build2(plans, in_group=KERNEL_IG, out_group=KERNEL_OG)
    pairs_max = max(len(pairs) for pairs, _ in plans)
    in_maps = _prep_inputs2(
        hidden_states, word_ids, plans, pairs_max, in_group=KERNEL_IG
    )
    res = bass_utils.run_bass_kernel_spmd(nc, in_maps, core_ids=list(range(N_CORES)))
    out = np.empty((B, NUM_WORDS, H), np.float32)
    OG = KERNEL_OG
    for c in range(N_CORES):
        # y is partition-major: [SPC*(NW//OG), P, OG*H]; word index of
        # element (slot, jg, p, a) is (jg*OG + a)*P + p.
        yc = np.asarray(res.results[c]["y"], dtype=np.float32)
        yc = (
            yc.reshape(SPC, NW // OG, P, OG, H)
            .transpose(0, 1, 3, 2, 4)
            .reshape(SPC, NUM_WORDS, H)
        )
        for slot in range(SPC):
            out[c * SPC + slot] = yc[slot]
    return out



# revision 18
# speedup vs baseline: 1.0236x; 1.0236x over previous
"""Segment-mean (word-pooling) kernel for Trainium2, 8 NeuronCores.

Problem: hidden_states [16, 4096, 768] f32, word_ids [16, 4096] i32
(non-decreasing per row, -1 = special token). Output [16, 2048, 768] f32:
mean of each word's subword embeddings; words with no tokens -> 0.

Strategy: pure data parallelism, 2 samples per core. Per sample, the
segment-mean is computed as a banded one-hot matmul on the PE:
  out[w, h] = sum_s onehot[s, w] * (1/count[w]) * x[s, h]
Tokens are processed in 32 k-tiles of 128; since word ids are
non-decreasing, each k-tile only touches a <=128-wide band of words, so
each k-tile contributes 1-2 matmuls into 128-word output windows
accumulated in PSUM. The one-hot (scaled by per-token reciprocal counts,
computed on host) is built on the vector engine with a single fused
is_equal*mult tensor_scalar op per k-tile against an iota ramp.

The SPMD program is identical on all 8 cores; the (k-tile, window)
pair structure is the union over samples, so per-core data that doesn't
touch a scheduled pair just contributes a zero one-hot block.
"""

import numpy as np

B, S, H = 16, 4096, 768
NUM_WORDS = S // 2  # 2048
N_CORES = 8
SPC = B // N_CORES  # samples per core = 2
P = 128
KT = S // P  # 32 k-tiles per sample
NW = NUM_WORDS // P  # 16 output windows per sample
NSPLITS = ((0, 512), (512, 768))  # matmul free-dim splits of H


def _plan(word_ids: np.ndarray):
    """Per-slot union plan. For each slot (0/1) and k-tile t: the window
    span [minwin, maxwin] over that slot's 8 samples; per window j the
    sorted member k-tiles. Returns (spans, members) per slot."""
    word_ids = np.minimum(word_ids, NUM_WORDS - 1)
    plans = []
    for slot in range(SPC):
        wid = word_ids[slot::SPC]  # the 8 samples this slot sees
        minwin = np.full(KT, NW, np.int64)
        maxwin = np.full(KT, -1, np.int64)
        for b in range(wid.shape[0]):
            row = wid[b]
            for t in range(KT):
                w = row[t * P : (t + 1) * P]
                w = w[w >= 0]
                if w.size:
                    minwin[t] = min(minwin[t], w.min() // P)
                    maxwin[t] = max(maxwin[t], w.max() // P)
        members = {j: [] for j in range(NW)}
        spans = []
        for t in range(KT):
            if maxwin[t] < 0:  # no valid token anywhere (can't happen)
                spans.append((0, 0))
                continue
            spans.append((int(minwin[t]), int(maxwin[t])))
            for j in range(int(minwin[t]), int(maxwin[t]) + 1):
                members[j].append(t)
        plans.append((spans, members))
    return plans


def _liveness(plans, in_group):
    """Max number of simultaneously-live x DMA groups / onehot tiles over
    the per-window emission order, across slots. A pool needs at least
    this many bufs or slot reuse can deadlock the DMA ring."""
    max_live_g, max_live_oh = 0, 0
    for spans, members in plans:
        first_g, last_g, first_oh, last_oh = {}, {}, {}, {}
        for j in range(NW):
            for t in members[j]:
                g = t // in_group
                first_g.setdefault(g, j)
                last_g[g] = j
                first_oh.setdefault(t, j)
                last_oh[t] = j
        for j in range(NW):
            live_g = sum(1 for g in first_g if first_g[g] <= j <= last_g[g])
            live_oh = sum(1 for t in first_oh if first_oh[t] <= j <= last_oh[t])
            max_live_g = max(max_live_g, live_g)
            max_live_oh = max(max_live_oh, live_oh)
    return max_live_g, max_live_oh


def _recip_counts(word_ids: np.ndarray) -> np.ndarray:
    """Per-token 1/count(word) as f32; 0 for special (-1) tokens."""
    r = np.zeros((B, S), np.float32)
    for b in range(B):
        wid = word_ids[b]
        valid = wid >= 0
        counts = np.bincount(wid[valid], minlength=NUM_WORDS)
        r[b, valid] = (1.0 / counts[wid[valid]]).astype(np.float32)
    return r


def _build(
    plans,
    reps=1,
    dyn_reps=1,
    do_mm=True,
    do_out=True,
    do_in=True,
    do_oh=True,
    do_wid=True,
    x_bufs=8,
    oh_bufs=8,
    ev_bufs=4,
    ps_bufs=3,
    in_group=4,
    out_group=2,
    in_alt=False,
    out_engine="scalar",
    in_dtype="f16",
    out_dtype="f16",
    n_splits=None,
    ev_engine="scalar",
    in_layout="rowmajor",
    staggered=False,
):
    """Build + compile the SPMD Bass program. reps>1 unrolls the whole
    body; dyn_reps>1 wraps it in a hardware For loop — both only used
    for amortized wall-clock timing. do_* flags ablate kernel stages
    for benchmarking (outputs are wrong when any is False)."""
    from contextlib import nullcontext
    import concourse.bacc as bacc
    import concourse.tile as tile
    from concourse import mybir

    nc = bacc.Bacc(
        "TRN2",
        target_bir_lowering=False,
        debug=False,
        enable_asserts=False,
        num_devices=N_CORES,
    )
    f32 = mybir.dt.float32
    fin = mybir.dt.float16 if in_dtype == "f16" else f32
    fout = mybir.dt.float16 if out_dtype == "f16" else f32
    if in_layout == "pmajor":
        x = nc.dram_tensor(
            "x", [SPC * KT // in_group, P, in_group * H], fin, kind="ExternalInput"
        ).ap()
    else:
        x = nc.dram_tensor("x", [SPC * S, H], fin, kind="ExternalInput").ap()
    widf = nc.dram_tensor("widf", [SPC, P, KT], f32, kind="ExternalInput").ap()
    rcp = nc.dram_tensor("rcp", [SPC, P, KT], f32, kind="ExternalInput").ap()
    y = nc.dram_tensor("y", [SPC * NUM_WORDS, H], fout, kind="ExternalOutput").ap()

    IOTA_W = NUM_WORDS + 2 * P  # ramp long enough for any window pair
    max_span = max(
        (jhi - jlo + 1) for spans, _ in plans for (jlo, jhi) in spans
    )

    # Size pools from plan liveness; degenerate plans (heavily overlapping
    # window k-ranges) fall back to a bounded reload mode, otherwise pool
    # slot reuse can deadlock the DMA ring.
    in_b = 2 if in_dtype == "f16" else 4
    live_g, live_oh = _liveness(plans, in_group)
    need_x, need_oh = live_g + 3, live_oh + 3
    x_bytes = need_x * in_group * H * in_b
    oh_bytes = need_oh * max_span * P * in_b
    safe = x_bytes + oh_bytes > 150 * 1024
    if not safe:
        x_bufs = max(x_bufs, need_x)
        oh_bufs = max(oh_bufs, need_oh)

    with tile.TileContext(nc) as tc:
        with (
            tc.tile_pool(name="const", bufs=1) as const_pool,
            tc.tile_pool(name="xin", bufs=x_bufs) as x_pool,
            tc.tile_pool(name="oh", bufs=oh_bufs) as oh_pool,
            tc.tile_pool(name="ev", bufs=ev_bufs) as ev_pool,
            tc.tile_pool(name="psum", bufs=ps_bufs, space="PSUM") as psum_pool,
        ):
            iota_i = const_pool.tile([P, IOTA_W], mybir.dt.int32)
            nc.gpsimd.iota(iota_i[:], pattern=[[1, IOTA_W]], base=0, channel_multiplier=0)
            iota_f = const_pool.tile([P, IOTA_W], f32)
            nc.vector.tensor_copy(out=iota_f[:], in_=iota_i[:])

            IG, OG = in_group, out_group
            out_eng = nc.sync if out_engine == "sync" else nc.scalar
            ev_eng = nc.vector if ev_engine == "vector" else nc.scalar
            splits = NSPLITS if n_splits is None else n_splits

            def emit(rep):
                for slot in range(SPC):
                    spans, members = plans[slot]
                    if do_wid:
                        wid_t = const_pool.tile(
                            [P, KT], f32, name=f"wid_{rep}_{slot}", tag=f"wid{slot}"
                        )
                        nc.scalar.dma_start(out=wid_t[:], in_=widf[slot, :, :])
                        rcp_t = const_pool.tile(
                            [P, KT], f32, name=f"rcp_{rep}_{slot}", tag=f"rcp{slot}"
                        )
                        nc.scalar.dma_start(out=rcp_t[:], in_=rcp[slot, :, :])

                    xg_tiles = {}
                    oh_tiles = {}

                    def get_x(t):
                        g, a = divmod(t, IG)
                        if g not in xg_tiles:
                            xt = x_pool.tile(
                                [P, IG, H], fin, name=f"xt_{rep}_{slot}_{g}", tag="xt"
                            )
                            if do_in:
                                if in_layout == "pmajor":
                                    src = x[slot * (KT // IG) + g, :, :].rearrange(
                                        "p (a h) -> p a h", a=IG
                                    )
                                else:
                                    r0 = slot * S + g * IG * P
                                    src = x[r0 : r0 + IG * P, :].rearrange(
                                        "(a p) h -> p a h", p=P
                                    )
                                eng = (
                                    nc.scalar
                                    if (in_alt and g % 2 == 1)
                                    else nc.sync
                                )
                                eng.dma_start(out=xt[:], in_=src)
                            xg_tiles[g] = xt
                        return xg_tiles[g][:, t % IG, :]

                    def get_oh(t):
                        if t not in oh_tiles:
                            jlo, jhi = spans[t]
                            wspan = (jhi - jlo + 1) * P
                            oh = oh_pool.tile(
                                [P, max_span * P],
                                fin,
                                name=f"oh_{rep}_{slot}_{t}",
                                tag="oh",
                            )
                            nc.vector.tensor_scalar(
                                out=oh[:, :wspan],
                                in0=iota_f[:, jlo * P : jlo * P + wspan],
                                scalar1=wid_t[:, t : t + 1],
                                scalar2=rcp_t[:, t : t + 1],
                                op0=mybir.AluOpType.is_equal,
                                op1=mybir.AluOpType.mult,
                            )
                            oh_tiles[t] = oh
                        return oh_tiles[t]

                    og_tile = [None]

                    for j in range(NW):
                        if (do_mm or do_out) and j % OG == 0:
                            og_tile[0] = ev_pool.tile(
                                [P, OG, H], fout, name=f"out_{rep}_{slot}_{j}", tag="out"
                            )
                        out_sb = og_tile[0][:, j % OG, :] if (do_mm or do_out) else None
                        ks = members[j]
                        if not do_mm:
                            for t in ks:
                                if do_in:
                                    get_x(t)
                                if do_oh:
                                    get_oh(t)
                        if not ks:
                            if out_sb is not None:
                                nc.vector.memset(out_sb, 0.0)
                        elif not do_mm:
                            if do_out:
                                nc.gpsimd.memset(out_sb, 0.0)
                        else:
                            ps = psum_pool.tile(
                                [P, H], f32, name=f"ps_{rep}_{slot}_{j}", tag="ps"
                            )
                            for ki, t in enumerate(ks):
                                if safe:
                                    xs = x_pool.tile(
                                        [P, 1, H], fin,
                                        name=f"xs_{rep}_{slot}_{j}_{t}", tag="xt",
                                    )
                                    r0 = slot * S + t * P
                                    nc.sync.dma_start(
                                        out=xs[:], in_=x[r0 : r0 + P, :].rearrange(
                                            "(a p) h -> p a h", p=P
                                        )
                                    )
                                    xt = xs[:, 0, :]
                                    oh = oh_pool.tile(
                                        [P, P], fin,
                                        name=f"ohs_{rep}_{slot}_{j}_{t}", tag="oh",
                                    )
                                    nc.vector.tensor_scalar(
                                        out=oh[:, :],
                                        in0=iota_f[:, j * P : (j + 1) * P],
                                        scalar1=wid_t[:, t : t + 1],
                                        scalar2=rcp_t[:, t : t + 1],
                                        op0=mybir.AluOpType.is_equal,
                                        op1=mybir.AluOpType.mult,
                                    )
                                    off = 0
                                else:
                                    xt = get_x(t)
                                    oh = get_oh(t)
                                    off = (j - spans[t][0]) * P
                                for lo, hi in splits:
                                    nc.tensor.matmul(
                                        out=ps[:, lo:hi],
                                        lhsT=oh[:, off : off + P],
                                        rhs=xt[:, lo:hi],
                                        start=(ki == 0),
                                        stop=(ki == len(ks) - 1),
                                    )
                            if ev_engine == "vector":
                                nc.vector.tensor_copy(out=out_sb, in_=ps[:])
                            else:
                                nc.scalar.copy(out=out_sb, in_=ps[:])
                        if do_out and j % OG == OG - 1:
                            r0 = slot * NUM_WORDS + (j - OG + 1) * P
                            dst = y[r0 : r0 + OG * P, :].rearrange(
                                "(a p) h -> p a h", p=P
                            )
                            oe = (
                                (nc.scalar if (j // OG) % 2 == 0 else nc.sync)
                                if out_engine == "alt"
                                else out_eng
                            )
                            oe.dma_start(out=dst, in_=og_tile[0][:])

            loop_cm = (
                tc.For_i(0, dyn_reps, 1, staggered_reset=staggered)
                if dyn_reps > 1
                else nullcontext()
            )
            with loop_cm:
                for rep in range(reps):
                    emit(rep)

    nc.compile()
    return nc


def _prep_inputs(hidden_states, word_ids, in_dtype="f16", in_layout="rowmajor",
                 in_group=4):
    np_in = np.float16 if in_dtype == "f16" else np.float32
    hs = np.ascontiguousarray(np.asarray(hidden_states, dtype=np_in))
    wid = np.minimum(np.asarray(word_ids, dtype=np.int32), NUM_WORDS - 1)
    assert hs.shape == (B, S, H) and wid.shape == (B, S)
    r = _recip_counts(wid)
    # [B, S] -> [B, P, KT]: element (p, t) = token t*P + p
    widf = np.ascontiguousarray(
        wid.astype(np.float32).reshape(B, KT, P).transpose(0, 2, 1)
    )
    rt = np.ascontiguousarray(r.reshape(B, KT, P).transpose(0, 2, 1))
    in_maps = []
    for c in range(N_CORES):
        sl = slice(c * SPC, (c + 1) * SPC)
        if in_layout == "pmajor":
            IG = in_group
            xc = np.ascontiguousarray(
                hs[sl]
                .reshape(SPC, KT // IG, IG, P, H)
                .transpose(0, 1, 3, 2, 4)
                .reshape(SPC * KT // IG, P, IG * H)
            )
        else:
            xc = hs[sl].reshape(SPC * S, H)
        in_maps.append({"x": xc, "widf": widf[sl], "rcp": rt[sl]})
    return in_maps


def _plan2(word_ids: np.ndarray):
    """Per-slot union plan, pair-flattened. For each slot: the ordered pair
    list [(j, t), ...] (windows ascending, k-tiles ascending within a
    window) and members[j] = sorted k-tiles feeding window j. The pair list
    is the union over the 8 samples mapped to that slot, so one SPMD
    program fits all cores; blocks a core's sample doesn't touch produce
    all-zero one-hots and contribute nothing."""
    wid = np.minimum(np.asarray(word_ids, np.int32), NUM_WORDS - 1)
    plans = []
    for slot in range(SPC):
        w = wid[slot::SPC]
        members = {j: [] for j in range(NW)}
        for t in range(KT):
            seg = w[:, t * P : (t + 1) * P]
            v = seg[seg >= 0]
            jlo, jhi = int(v.min()) // P, int(v.max()) // P
            for j in range(jlo, jhi + 1):
                members[j].append(t)
        pairs = [(j, t) for j in range(NW) for t in members[j]]
        plans.append((pairs, members))
    return plans


def _x_liveness2(members, in_group):
    """Max simultaneously-live x groups over the window emission order."""
    first_g, last_g = {}, {}
    for j in range(NW):
        for t in members[j]:
            g = t // in_group
            first_g.setdefault(g, j)
            last_g[g] = j
    return max(
        sum(1 for g in first_g if first_g[g] <= j <= last_g[g]) for j in range(NW)
    )


def _prep_inputs2(hidden_states, word_ids, plans, pairs_max, in_group=8):
    """Per-core inputs for _build2. shifted[slot][p, i] = word_id of token
    (t_i*P + p) minus j_i*128 for pair i=(j_i,t_i): in [0,128) iff the
    token's word is in pair i's window (specials/-1 and out-of-window
    tokens fall outside automatically). rcpw[slot][p, j] = 1/count(word
    j*128+p), 0 for empty words. x is partition-major: [group, p,
    IG*H] so each partition's bytes per load are one contiguous run."""
    hs = np.ascontiguousarray(np.asarray(hidden_states, dtype=np.float16))
    wid = np.minimum(np.asarray(word_ids, dtype=np.int32), NUM_WORDS - 1)
    assert hs.shape == (B, S, H) and wid.shape == (B, S)
    IG = in_group
    in_maps = []
    for c in range(N_CORES):
        shifted = np.full((SPC, P, pairs_max), -2048, np.float16)
        rcpw = np.zeros((SPC, P, NW), np.float32)
        for slot in range(SPC):
            pairs, _ = plans[slot]
            w = wid[c * SPC + slot]
            for i, (j, t) in enumerate(pairs):
                shifted[slot, :, i] = (w[t * P : (t + 1) * P] - j * P).astype(
                    np.float16
                )
            valid = w >= 0
            counts = np.bincount(w[valid], minlength=NUM_WORDS)
            r = np.zeros(NUM_WORDS, np.float32)
            nz = counts > 0
            r[nz] = 1.0 / counts[nz]
            rcpw[slot] = r.reshape(NW, P).T
        xc = np.ascontiguousarray(
            hs[c * SPC : (c + 1) * SPC]
            .reshape(SPC, KT // IG, IG, P, H)
            .transpose(0, 1, 3, 2, 4)
            .reshape(SPC * (KT // IG), P, IG * H)
        )
        in_maps.append({"x": xc, "shifted": shifted, "rcpw": rcpw})
    return in_maps


def _build2(
    plans,
    reps=1,
    dyn_reps=1,
    do_mm=True,
    do_out=True,
    do_in=True,
    do_oh=True,
    x_bufs=6,
    oh_bufs=3,
    ev_bufs=4,
    ps_bufs=4,
    in_group=8,
    out_group=4,
    out_engine="scalar",
    ev_engine="both",
    oh_chunk=16,
    in_alt=False,
    staggered=False,
):
    """v2: one-hot blocks for ALL pairs of a slot are built with a single
    tensor_tensor is_equal against a broadcast 0..127 ramp; 1/count scaling
    is applied per word row during PSUM evacuation (per-partition scale),
    so no per-k-tile vector ops remain."""
    from contextlib import nullcontext
    import concourse.bacc as bacc
    import concourse.tile as tile
    from concourse import mybir

    nc = bacc.Bacc(
        "TRN2",
        target_bir_lowering=False,
        debug=False,
        enable_asserts=False,
        num_devices=N_CORES,
    )
    f32 = mybir.dt.float32
    f16 = mybir.dt.float16
    i32 = mybir.dt.int32

    IG, OG = in_group, out_group
    pairs_max = max(len(pairs) for pairs, _ in plans)
    x = nc.dram_tensor(
        "x", [SPC * (KT // IG), P, IG * H], f16, kind="ExternalInput"
    ).ap()
    shifted = nc.dram_tensor(
        "shifted", [SPC, P, pairs_max], f16, kind="ExternalInput"
    ).ap()
    rcpw = nc.dram_tensor("rcpw", [SPC, P, NW], f32, kind="ExternalInput").ap()
    y = nc.dram_tensor(
        "y", [SPC * (NW // OG), P, OG * H], f16, kind="ExternalOutput"
    ).ap()
    live = max(_x_liveness2(members, IG) for _, members in plans)
    x_bufs = max(x_bufs, live + 2)
    assert x_bufs * IG * H * 2 * P <= 14 * 1024 * 1024, "x pool too big"

    with tile.TileContext(nc) as tc:
        with (
            tc.tile_pool(name="const", bufs=1) as const_pool,
            tc.tile_pool(name="sw", bufs=4) as sw_pool,
            tc.tile_pool(name="xin", bufs=x_bufs) as x_pool,
            tc.tile_pool(name="oh", bufs=oh_bufs) as oh_pool,
            tc.tile_pool(name="ev", bufs=ev_bufs) as ev_pool,
            tc.tile_pool(name="psum", bufs=ps_bufs, space="PSUM") as psum_pool,
        ):
            ramp_i = const_pool.tile([P, P], i32)
            nc.gpsimd.iota(ramp_i[:], pattern=[[1, P]], base=0, channel_multiplier=0)
            ramp_f = const_pool.tile([P, P], f16)
            nc.vector.tensor_copy(out=ramp_f[:], in_=ramp_i[:])

            def emit(rep):
                for slot in range(SPC):
                    pairs, members = plans[slot]
                    npair = len(pairs)
                    sh_t = sw_pool.tile(
                        [P, pairs_max], f16, name=f"sh_{rep}_{slot}", tag="sh"
                    )
                    rw_t = sw_pool.tile(
                        [P, NW], f32, name=f"rw_{rep}_{slot}", tag="rw"
                    )
                    if do_oh or do_mm:
                        nc.scalar.dma_start(out=sh_t[:], in_=shifted[slot, :, :])
                        nc.scalar.dma_start(out=rw_t[:], in_=rcpw[slot, :, :])

                    oh_all = oh_pool.tile(
                        [P, pairs_max * P], f16, name=f"oh_{rep}_{slot}", tag="oh"
                    )
                    if do_oh or do_mm:
                        for c0 in range(0, npair, oh_chunk):
                            c1 = min(c0 + oh_chunk, npair)
                            nchunk = c1 - c0
                            nc.vector.tensor_tensor(
                                out=oh_all[:, c0 * P : c1 * P].rearrange(
                                    "p (i w) -> p i w", w=P
                                ),
                                in0=sh_t[:, c0:c1].unsqueeze(2).broadcast_to(
                                    (P, nchunk, P)
                                ),
                                in1=ramp_f[:].unsqueeze(1).broadcast_to(
                                    (P, nchunk, P)
                                ),
                                op=mybir.AluOpType.is_equal,
                            )

                    xg_tiles = {}

                    def get_x(t):
                        g, a = divmod(t, IG)
                        if g not in xg_tiles:
                            xt = x_pool.tile(
                                [P, IG, H], f16, name=f"xt_{rep}_{slot}_{g}", tag="xt"
                            )
                            if do_in:
                                src = x[slot * (KT // IG) + g, :, :].rearrange(
                                    "p (a h) -> p a h", a=IG
                                )
                                eng = (
                                    nc.scalar
                                    if (in_alt and g % 2 == 1)
                                    else nc.sync
                                )
                                eng.dma_start(out=xt[:], in_=src)
                            xg_tiles[g] = xt
                        return xg_tiles[g][:, t % IG, :]

                    og_tile = [None]
                    pair_idx = 0
                    for j in range(NW):
                        if (do_mm or do_out) and j % OG == 0:
                            og_tile[0] = ev_pool.tile(
                                [P, OG, H], f16, name=f"out_{rep}_{slot}_{j}",
                                tag="out",
                            )
                        out_sb = (
                            og_tile[0][:, j % OG, :] if (do_mm or do_out) else None
                        )
                        ks = members[j]
                        if not do_mm:
                            pair_idx += len(ks)
                            if do_in:
                                for t in ks:
                                    get_x(t)
                            if do_out and out_sb is not None:
                                nc.gpsimd.memset(out_sb, 0.0)
                        elif not ks:
                            nc.vector.memset(out_sb, 0.0)
                        else:
                            ps = psum_pool.tile(
                                [P, H], f32, name=f"ps_{rep}_{slot}_{j}", tag="ps"
                            )
                            for ki, t in enumerate(ks):
                                xt = get_x(t)
                                lhsT = oh_all[:, pair_idx * P : (pair_idx + 1) * P]
                                pair_idx += 1
                                for lo, hi in NSPLITS:
                                    nc.tensor.matmul(
                                        out=ps[:, lo:hi],
                                        lhsT=lhsT,
                                        rhs=xt[:, lo:hi],
                                        start=(ki == 0),
                                        stop=(ki == len(ks) - 1),
                                    )
                            scale = rw_t[:, j : j + 1]
                            use_vec = ev_engine == "vector" or (
                                ev_engine == "both" and j % 2 == 0
                            )
                            if use_vec:
                                nc.vector.tensor_scalar(
                                    out=out_sb,
                                    in0=ps[:],
                                    scalar1=scale,
                                    scalar2=None,
                                    op0=mybir.AluOpType.mult,
                                )
                            else:
                                nc.scalar.activation(
                                    out_sb,
                                    ps[:],
                                    mybir.ActivationFunctionType.Copy,
                                    scale=scale,
                                )
                        if do_out and j % OG == OG - 1:
                            dst = y[slot * (NW // OG) + j // OG, :, :].rearrange(
                                "p (a h) -> p a h", a=OG
                            )
                            oe = (
                                (nc.scalar if (j // OG) % 2 == 0 else nc.sync)
                                if out_engine == "alt"
                                else (nc.sync if out_engine == "sync" else nc.scalar)
                            )
                            oe.dma_start(out=dst, in_=og_tile[0][:])

            loop_cm = (
                tc.For_i(0, dyn_reps, 1, staggered_reset=staggered)
                if dyn_reps > 1
                else nullcontext()
            )
            with loop_cm:
                for rep in range(reps):
                    emit(rep)

    nc.compile()
    return nc


KERNEL_IG = 8
KERNEL_OG = 4


def kernel(hidden_states, word_ids):
    import concourse.bass_utils as bass_utils

    wid = np.asarray(word_ids, dtype=np.int32)
    plans = _plan2(wid)
    nc = _build2(plans, in_group=KERNEL_IG, out_group=KERNEL_OG)
    pairs_max = max(len(pairs) for pairs, _ in plans)
    in_maps = _prep_inputs2(
        hidden_states, word_ids, plans, pairs_max, in_group=KERNEL_IG
    )
    res = bass_utils.run_bass_kernel_spmd(nc, in_maps, core_ids=list(range(N_CORES)))
    out = np.empty((B, NUM_WORDS, H), np.float32)
    OG = KERNEL_OG
    for c in range(N_CORES):
        # y is partition-major: [SPC*(NW//OG), P, OG*H]; word index of
        # element (slot, jg, p, a) is (jg*OG + a)*P + p.
        yc = np.asarray(res.results[c]["y"], dtype=np.float32)
        yc = (
            yc.reshape(SPC, NW // OG, P, OG, H)
            .transpose(0, 1, 3, 2, 4)
            .reshape(SPC, NUM_WORDS, H)
        )
        for slot in range(SPC):
            out[c * SPC + slot] = yc[slot]
    return out

